# revision 30
# baseline (speedup 1.0000x reference)
"""Trainium2 Bass kernel for nn_DenseFilterExpansion.

Computes out[b, f, t] = x[b, 0, t] * w[f, t] + bias[f, t] for
x: (128, 1, 4096), w/bias: (256, 4096)  ->  out: (128, 256, 4096) fp32.

The kernel is HBM-write-bound, so the device computes and stores the
output in bf16 (half the write bytes of fp32); the host widens the
result back to fp32. End-to-end relative error ~3e-3 against the fp32
reference (harness gate is 2e-2): one fp8-pair rounding of x, one bf16
rounding of w, one bf16 rounding of the product.

Per core (data-parallel over batch, 16 batches/core):
  - x ships as a host-built Dekker pair hi+lo in fp8-e4m3 (hi + lo == x
    to ~8e-4) resident in SBUF on partitions 0-15, plus a tiny host-
    built fp8 selection matrix sel[k, (bi, r, p)] = (k == bi).
  - Per batch, a DoubleRow fp8 matmul with lhsT = sel[:, bi] both
    selects batch row bi (matmul operands must sit at base partition 0)
    and sums its hi/lo pair while broadcasting it across the 128 output
    partitions into PSUM (fp32); ScalarE (ACT) cast-copies each PSUM
    half to a bf16 SBUF tile xb. Keeping every mid-kernel DMA on HWDGE
    (no SWDGE) avoids the known SDMA-engine-15 straggler mode.
  - w stays resident as two (128, 4096) bf16 tiles; VectorE multiplies
    w_c * xb (tensor_tensor, all-bf16 SBUF operands -> DVE 2x perf
    mode, ~2.3 us per (batch, f-chunk) tile, 73 us total).
  - Both f-chunks of a batch land in one (128, 8192) bf16 tile, stored
    with a single 2 MiB HWDGE DMA (3D DRAM access pattern, two
    contiguous 8 KiB runs per partition), alternating the SP/ACT rings.

Engine budget per core: PE ~79 us (pipelined to ~70 us span), ACT
~63 us, DVE ~73 us, DMA ~86 us of per-SDMA-engine busy time (each of
the 16 engines moves 1/16th of ~36 MB at ~26 GB/s) -- the store stream
is the roofline. Measured ~110 us vs 185 us for the fp32 variant.

Optimization notes from the 2026-08-10 session (alternatives measured
WORSE than this structure; see memory/trn2-densefilter-findings.md):
  - The minimal-DMA-count structure here (21 DMAs) sustains the highest
    per-engine SDMA rate (~26.4 GB/s).  Variants with split w loads,
    per-quarter stores, or a fine-grained first-batch ramp measured
    107-122 us: extra small DMAs shave the steady-state rate and can
    trigger an SDMA-engine-15 straggler (~8% slow for the whole run,
    +10-15 us on the end barrier) -- seen with all stores on one ring
    or with bursts of piece-stores.
  - Mixed fp8 output (6/16 batches via SWDGE cast-during-DMA, exact
    RTN, rel err 1.65e-2) cuts stores 32->26 MiB but measured 113-116
    us: Q7 descriptor generation (~3 us/DMA) serializes behind DVE and
    starves the HWDGE streams.
  - PE warm-up bursts are useless: K=16 broadcasts light only 16 of
    128 PE rows and never trip the HAM clock gate; MMs stay at the
    1.2 GHz cold pace regardless.
  - Fixed NEFF overhead is ~10 us in-window (prologue ~2.7 us before
    the first data packet + ~8.5 us epilogue: DMA-sem receipt, tile
    barrier, compiler-emitted clear of all 249 semaphores).  An empty
    kernel measures 15.3 us.
"""

import numpy as np
import ml_dtypes

import concourse.bacc as bacc
import concourse.bass as bass
import concourse.mybir as mybir
import concourse.tile as tile
from concourse.bass_utils import run_bass_kernel_spmd

N_CORES = 8
B_FULL = 128
F = 256
T = 4096
BS = B_FULL // N_CORES  # batches per core = 16
P = 128                 # partitions
FP = F // P             # f-chunks = 2
TH = 2048               # psum tile width (4 banks)
MM_N = 512              # matmul free dim (one PSUM bank, ISA cap)
NH = T // TH            # 2 psum halves per batch

_nc_cache: dict = {}


def _build(with_bias: bool) -> bass.Bass:
    f32 = mybir.dt.float32
    bf16 = mybir.dt.bfloat16
    nc = bacc.Bacc("TRN2", debug=False)

    f8 = mybir.dt.float8e4
    # x ships as an exact-ish Dekker pair hi+lo in fp8-e4m3 (combined
    # representation error ~8e-4, better than one bf16 rounding). The
    # DoubleRow matmul sums the pair while broadcasting, at 2x PE row
    # rate.
    x_d = nc.dram_tensor("x2", [BS, 2 * T], f8, kind="ExternalInput")
    sel_d = nc.dram_tensor("sel2", [BS, BS * 2 * P], f8, kind="ExternalInput")
    w_d = nc.dram_tensor("w", [F, T], bf16, kind="ExternalInput")
    b_d = (
        nc.dram_tensor("bvec", [F, T], bf16, kind="ExternalInput")
        if with_bias
        else None
    )
    o_d = nc.dram_tensor("out", [BS, F, T], bf16, kind="ExternalOutput")

    with tile.TileContext(nc) as tc:
        with (
            tc.tile_pool(name="const", bufs=1) as cpool,
            tc.tile_pool(name="xbp", bufs=4) as xpool,
            tc.tile_pool(name="outp", bufs=4) as opool,
            tc.tile_pool(name="psum", bufs=2, space="PSUM") as ppool,
        ):
            # Selection matrix (host-built): sel[k, (bi, r, p)] = (k ==
            # bi). A DoubleRow K=16x2 fp8 matmul with lhsT = sel[:, bi]
            # broadcasts (and sums) the x hi/lo pair of row bi across
            # the 128 output partitions, reading the resident x block at
            # base partition 0 (HW requires matmul operands at base
            # partition 0/32/64). This keeps all mid-kernel DMA off
            # SWDGE (whose descriptor-ring traffic makes SDMA engine 15
            # a straggler).
            # x hi/lo block resident on partitions 0-15 (one 128 KiB
            # HWDGE DMA). The first matmul is gated by the completion
            # semaphores of x2 and sel, so each goes FIRST on its own
            # ring (x2 on SP, sel on ACT) to complete in parallel.
            x_sb = cpool.tile([BS, 2 * T], f8, tag="x2")
            nc.sync.dma_start(out=x_sb[:], in_=x_d[:, :])
            x_rt = x_sb[0:BS, :].rearrange("k (r t) -> k r t", r=2)

            sel = cpool.tile([BS, BS * 2 * P], f8, tag="sel")
            nc.scalar.dma_start(out=sel[:], in_=sel_d[:, :])

            w_sb = {}
            b_sb = {}
            for c in range(FP):
                wt = cpool.tile([P, T], bf16, tag=f"w{c}", name=f"w{c}")
                # Both w tiles on the ACT ring: keeps them off the DMA
                # semaphore lane the first matmuls wait on (sel2+x2 on
                # SP), so PE starts ~4 us earlier.  (Splitting these
                # loads into halves/quarters -- or splitting the first/
                # last batch stores per chunk -- reproducibly triggers
                # the SDMA-engine-15 straggler: 2/2 runs at ~120 us vs
                # 4/4 clean runs for this exact shape.)
                nc.scalar.dma_start(out=wt[:], in_=w_d[c * P : (c + 1) * P, :])
                w_sb[c] = wt
                if with_bias:
                    bt = cpool.tile([P, T], bf16, tag=f"b{c}", name=f"b{c}")
                    nc.gpsimd.dma_start(
                        out=bt[:], in_=b_d[c * P : (c + 1) * P, :]
                    )
                    b_sb[c] = bt

            for bi in range(BS):
                # Broadcast x row bi across 128 partitions: selection
                # matmul into PSUM (fp32), then ACT cast-copies to bf16
                # SBUF.
                xb = xpool.tile([P, T], bf16, tag="xb", name=f"xb{bi}")
                for h in range(NH):
                    ps = ppool.tile([P, TH], f32, tag="ps", name=f"ps{bi}_{h}")
                    for j in range(TH // MM_N):
                        col = h * TH + j * MM_N
                        nc.tensor.matmul(
                            ps[:, j * MM_N : (j + 1) * MM_N],
                            sel[0:BS, bi * 2 * P : (bi + 1) * 2 * P].rearrange(
                                "k (r p) -> k r p", r=2
                            ),
                            x_rt[:, :, col : col + MM_N],
                            start=True,
                            stop=True,
                            perf_mode=mybir.MatmulPerfMode.DoubleRow,
                        )
                    nc.scalar.copy(
                        out=xb[:, h * TH : (h + 1) * TH], in_=ps[:]
                    )
                # Both f-chunks land in one [128, 2T] tile -> a single
                # 2 MiB store per batch (3D DRAM AP: two contiguous 8 KiB
                # runs per partition).
                ot = opool.tile([P, FP * T], bf16, tag="ot", name=f"ot{bi}")
                # Ramp batches (0-2) multiply in 2048-col h-major slices
                # (h0c0, h0c1, h1c0, h1c1): the first DVE op then only
                # needs ACT's h0 copy (~14.8 us) plus the w0 sem (~13.5
                # us -- the serial ACT-ring w loads fire w0's sem early;
                # do NOT parallelize w0/w1 across rings, packet round-
                # robin would delay BOTH sems to ~17 us), so batch 0's
                # store issues ~3 us earlier and the engine-limited
                # store stream starts sooner.  DVE-op granularity only;
                # the DMA shape stays byte-identical to the proven one.
                dw = TH if bi < 3 else T
                for k in range(T // dw):
                    for c in range(FP):
                        ds = slice(k * dw, (k + 1) * dw)
                        os_ = slice(c * T + k * dw, c * T + (k + 1) * dw)
                        # all-bf16 SBUF tensor_tensor -> DVE 2x perf mode
                        nc.vector.tensor_mul(
                            out=ot[:, os_], in0=w_sb[c][:, ds], in1=xb[:, ds]
                        )
                        if with_bias:
                            nc.vector.tensor_add(
                                out=ot[:, os_],
                                in0=ot[:, os_],
                                in1=b_sb[c][:, ds],
                            )
                # Alternate stores across both HWDGE rings.  (Splitting
                # batch 0's store per chunk measured 114.8 us vs 105.1
                # for this shape -- kept the single-store form.)
                ring = nc.sync if bi % 2 == 0 else nc.scalar
                ring.dma_start(
                    out=o_d[bi, :, :].rearrange("(c p) t -> p c t", p=P),
                    in_=ot[:].rearrange("p (c t) -> p c t", c=FP),
                )
    nc.finalize()
    return nc


def _get_nc(with_bias: bool) -> bass.Bass:
    if with_bias not in _nc_cache:
        _nc_cache[with_bias] = _build(with_bias)
    return _nc_cache[with_bias]


def _prepare(inputs: np.ndarray, w: np.ndarray, b: np.ndarray):
    """Host-side prep shared by kernel() and the traced test path."""
    bf = ml_dtypes.bfloat16
    f8 = ml_dtypes.float8_e4m3
    x = np.ascontiguousarray(inputs.reshape(B_FULL, T), dtype=np.float32)
    with_bias = bool(np.any(b))
    wb = np.ascontiguousarray(w).astype(bf)
    bb = np.ascontiguousarray(b).astype(bf) if with_bias else None

    # Exact-ish fp8 Dekker pair: hi + lo == x to ~8e-4 (fp32 sum).
    hi = x.astype(f8)
    lo = (x - hi.astype(np.float32)).astype(f8)
    x2 = np.stack([hi, lo], axis=1).reshape(B_FULL, 2 * T)

    sel = np.zeros((BS, BS, 2, P), dtype=f8)
    for bi in range(BS):
        sel[bi, bi, :, :] = 1.0
    sel = sel.reshape(BS, BS * 2 * P)

    nc = _get_nc(with_bias)
    in_maps = []
    for c in range(N_CORES):
        m = {
            "x2": np.ascontiguousarray(x2[c * BS : (c + 1) * BS]),
            "sel2": sel,
            "w": wb,
        }
        if with_bias:
            m["bvec"] = bb
        in_maps.append(m)
    return nc, in_maps


def _finish(res) -> np.ndarray:
    out = np.concatenate([np.asarray(r["out"]) for r in res.results], axis=0)
    return out.astype(np.float32)


def kernel(inputs: np.ndarray, w: np.ndarray, b: np.ndarray, **kw) -> np.ndarray:
    nc, in_maps = _prepare(inputs, w, b)
    res = run_bass_kernel_spmd(nc, in_maps, core_ids=list(range(N_CORES)))
    return _finish(res)


# revision 31
# speedup vs baseline: 1.0031x; 1.0031x over previous
"""Trainium2 Bass kernel for nn_DenseFilterExpansion.

Computes out[b, f, t] = x[b, 0, t] * w[f, t] + bias[f, t] for
x: (128, 1, 4096), w/bias: (256, 4096)  ->  out: (128, 256, 4096) fp32.

The kernel is HBM-write-bound, so the device computes and stores the
output in bf16 (half the write bytes of fp32); the host widens the
result back to fp32. End-to-end relative error ~3e-3 against the fp32
reference (harness gate is 2e-2): one fp8-pair rounding of x, one bf16
rounding of w, one bf16 rounding of the product.

Per core (data-parallel over batch, 16 batches/core):
  - x ships as a host-built Dekker pair hi+lo in fp8-e4m3 (hi + lo == x
    to ~8e-4) resident in SBUF on partitions 0-15, plus a tiny host-
    built fp8 selection matrix sel[k, (bi, r, p)] = (k == bi).
  - Per batch, a DoubleRow fp8 matmul with lhsT = sel[:, bi] both
    selects batch row bi (matmul operands must sit at base partition 0)
    and sums its hi/lo pair while broadcasting it across the 128 output
    partitions into PSUM (fp32); ScalarE (ACT) cast-copies each PSUM
    half to a bf16 SBUF tile xb. Keeping every mid-kernel DMA on HWDGE
    (no SWDGE) avoids the known SDMA-engine-15 straggler mode.
  - w stays resident as two (128, 4096) bf16 tiles; VectorE multiplies
    w_c * xb (tensor_tensor, all-bf16 SBUF operands -> DVE 2x perf
    mode, ~2.3 us per (batch, f-chunk) tile, 73 us total).
  - Both f-chunks of a batch land in one (128, 8192) bf16 tile, stored
    with a single 2 MiB HWDGE DMA (3D DRAM access pattern, two
    contiguous 8 KiB runs per partition), alternating the SP/ACT rings.

Engine budget per core: PE ~79 us (pipelined to ~70 us span), ACT
~63 us, DVE ~73 us, DMA ~86 us of per-SDMA-engine busy time (each of
the 16 engines moves 1/16th of ~36 MB at ~26 GB/s) -- the store stream
is the roofline. Measured ~110 us vs 185 us for the fp32 variant.

Optimization notes from the 2026-08-10 session (alternatives measured
WORSE than this structure; see memory/trn2-densefilter-findings.md):
  - The minimal-DMA-count structure here (21 DMAs) sustains the highest
    per-engine SDMA rate (~26.4 GB/s).  Variants with split w loads,
    per-quarter stores, or a fine-grained first-batch ramp measured
    107-122 us: extra small DMAs shave the steady-state rate and can
    trigger an SDMA-engine-15 straggler (~8% slow for the whole run,
    +10-15 us on the end barrier) -- seen with all stores on one ring
    or with bursts of piece-stores.
  - Mixed fp8 output (6/16 batches via SWDGE cast-during-DMA, exact
    RTN, rel err 1.65e-2) cuts stores 32->26 MiB but measured 113-116
    us: Q7 descriptor generation (~3 us/DMA) serializes behind DVE and
    starves the HWDGE streams.
  - PE warm-up bursts are useless: K=16 broadcasts light only 16 of
    128 PE rows and never trip the HAM clock gate; MMs stay at the
    1.2 GHz cold pace regardless.
  - Fixed NEFF overhead is ~10 us in-window (prologue ~2.7 us before
    the first data packet + ~8.5 us epilogue: DMA-sem receipt, tile
    barrier, compiler-emitted clear of all 249 semaphores).  An empty
    kernel measures 15.3 us.
"""

import numpy as np
import ml_dtypes

import concourse.bacc as bacc
import concourse.bass as bass
import concourse.mybir as mybir
import concourse.tile as tile
from concourse.bass_utils import run_bass_kernel_spmd

N_CORES = 8
B_FULL = 128
F = 256
T = 4096
BS = B_FULL // N_CORES  # batches per core = 16
P = 128                 # partitions
FP = F // P             # f-chunks = 2
TH = 2048               # psum tile width (4 banks)
MM_N = 512              # matmul free dim (one PSUM bank, ISA cap)
NH = T // TH            # 2 psum halves per batch

_nc_cache: dict = {}


def _build(with_bias: bool) -> bass.Bass:
    f32 = mybir.dt.float32
    bf16 = mybir.dt.bfloat16
    nc = bacc.Bacc("TRN2", debug=False)

    f8 = mybir.dt.float8e4
    # x ships as an exact-ish Dekker pair hi+lo in fp8-e4m3 (combined
    # representation error ~8e-4, better than one bf16 rounding). The
    # DoubleRow matmul sums the pair while broadcasting, at 2x PE row
    # rate.
    x_d = nc.dram_tensor("x2", [BS, 2 * T], f8, kind="ExternalInput")
    sel_d = nc.dram_tensor("sel2", [BS, BS * 2 * P], f8, kind="ExternalInput")
    w_d = nc.dram_tensor("w", [F, T], bf16, kind="ExternalInput")
    b_d = (
        nc.dram_tensor("bvec", [F, T], bf16, kind="ExternalInput")
        if with_bias
        else None
    )
    o_d = nc.dram_tensor("out", [BS, F, T], bf16, kind="ExternalOutput")

    with tile.TileContext(nc) as tc:
        with (
            tc.tile_pool(name="const", bufs=1) as cpool,
            tc.tile_pool(name="xbp", bufs=4) as xpool,
            tc.tile_pool(name="outp", bufs=4) as opool,
            tc.tile_pool(name="psum", bufs=2, space="PSUM") as ppool,
        ):
            # Selection matrix (host-built): sel[k, (bi, r, p)] = (k ==
            # bi). A DoubleRow K=16x2 fp8 matmul with lhsT = sel[:, bi]
            # broadcasts (and sums) the x hi/lo pair of row bi across
            # the 128 output partitions, reading the resident x block at
            # base partition 0 (HW requires matmul operands at base
            # partition 0/32/64). This keeps all mid-kernel DMA off
            # SWDGE (whose descriptor-ring traffic makes SDMA engine 15
            # a straggler).
            # x hi/lo block resident on partitions 0-15 (one 128 KiB
            # HWDGE DMA). The first matmul is gated by the completion
            # semaphores of x2 and sel, so each goes FIRST on its own
            # ring (x2 on SP, sel on ACT) to complete in parallel.
            x_sb = cpool.tile([BS, 2 * T], f8, tag="x2")
            nc.sync.dma_start(out=x_sb[:], in_=x_d[:, :])
            x_rt = x_sb[0:BS, :].rearrange("k (r t) -> k r t", r=2)

            sel = cpool.tile([BS, BS * 2 * P], f8, tag="sel")
            nc.scalar.dma_start(out=sel[:], in_=sel_d[:, :])

            w_sb = {}
            b_sb = {}
            for c in range(FP):
                wt = cpool.tile([P, T], bf16, tag=f"w{c}", name=f"w{c}")
                # Both w tiles on the ACT ring: keeps them off the DMA
                # semaphore lane the first matmuls wait on (sel2+x2 on
                # SP), so PE starts ~4 us earlier.  (Splitting these
                # loads into halves/quarters -- or splitting the first/
                # last batch stores per chunk -- reproducibly triggers
                # the SDMA-engine-15 straggler: 2/2 runs at ~120 us vs
                # 4/4 clean runs for this exact shape.)
                nc.scalar.dma_start(out=wt[:], in_=w_d[c * P : (c + 1) * P, :])
                w_sb[c] = wt
                if with_bias:
                    bt = cpool.tile([P, T], bf16, tag=f"b{c}", name=f"b{c}")
                    nc.gpsimd.dma_start(
                        out=bt[:], in_=b_d[c * P : (c + 1) * P, :]
                    )
                    b_sb[c] = bt

            for bi in range(BS):
                # Broadcast x row bi across 128 partitions: selection
                # matmul into PSUM (fp32), then ACT cast-copies to bf16
                # SBUF.
                xb = xpool.tile([P, T], bf16, tag="xb", name=f"xb{bi}")
                for h in range(NH):
                    ps = ppool.tile([P, TH], f32, tag="ps", name=f"ps{bi}_{h}")
                    for j in range(TH // MM_N):
                        col = h * TH + j * MM_N
                        nc.tensor.matmul(
                            ps[:, j * MM_N : (j + 1) * MM_N],
                            sel[0:BS, bi * 2 * P : (bi + 1) * 2 * P].rearrange(
                                "k (r p) -> k r p", r=2
                            ),
                            x_rt[:, :, col : col + MM_N],
                            start=True,
                            stop=True,
                            perf_mode=mybir.MatmulPerfMode.DoubleRow,
                        )
                    nc.scalar.copy(
                        out=xb[:, h * TH : (h + 1) * TH], in_=ps[:]
                    )
                # Both f-chunks land in one [128, 2T] tile -> a single
                # 2 MiB store per batch (3D DRAM AP: two contiguous 8 KiB
                # runs per partition).
                ot = opool.tile([P, FP * T], bf16, tag="ot", name=f"ot{bi}")
                # Ramp batches (0-2) multiply in 2048-col h-major slices
                # (h0c0, h0c1, h1c0, h1c1): the first DVE op then only
                # needs ACT's h0 copy (~14.8 us) plus the w0 sem (~13.5
                # us -- the serial ACT-ring w loads fire w0's sem early;
                # do NOT parallelize w0/w1 across rings, packet round-
                # robin would delay BOTH sems to ~17 us), so batch 0's
                # store issues ~3 us earlier and the engine-limited
                # store stream starts sooner.  DVE-op granularity only;
                # the DMA shape stays byte-identical to the proven one.
                dw = TH if bi < 3 else T
                for k in range(T // dw):
                    for c in range(FP):
                        ds = slice(k * dw, (k + 1) * dw)
                        os_ = slice(c * T + k * dw, c * T + (k + 1) * dw)
                        # all-bf16 SBUF tensor_tensor -> DVE 2x perf mode
                        nc.vector.tensor_mul(
                            out=ot[:, os_], in0=w_sb[c][:, ds], in1=xb[:, ds]
                        )
                        if with_bias:
                            nc.vector.tensor_add(
                                out=ot[:, os_],
                                in0=ot[:, os_],
                                in1=b_sb[c][:, ds],
                            )
                if bi == 0:
                    # Batch 0 stores as four 512 KiB quarter-DMAs on
                    # the otherwise-idle SP ring, one issued right
                    # after each DVE slice (subtile deps on the shared
                    # ot tile): the first store packet lands ~16.8 us
                    # instead of ~20.1, starting the engine-limited
                    # store stream ~3.3 us earlier.  Four SP pieces is
                    # inside the measured-safe envelope (8 was clean;
                    # bursts of 24+ on one ring trigger the engine-15
                    # straggler).
                    for k in range(NH):
                        for c in range(FP):
                            hs = slice(k * TH, (k + 1) * TH)
                            nc.sync.dma_start(
                                out=o_d[bi, c * P : (c + 1) * P, hs],
                                in_=ot[:, c * T + k * TH : c * T + (k + 1) * TH],
                            )
                else:
                    # Alternate stores across both HWDGE rings.
                    ring = nc.sync if bi % 2 == 0 else nc.scalar
                    ring.dma_start(
                        out=o_d[bi, :, :].rearrange("(c p) t -> p c t", p=P),
                        in_=ot[:].rearrange("p (c t) -> p c t", c=FP),
                    )
    nc.finalize()
    return nc


def _get_nc(with_bias: bool) -> bass.Bass:
    if with_bias not in _nc_cache:
        _nc_cache[with_bias] = _build(with_bias)
    return _nc_cache[with_bias]


def _prepare(inputs: np.ndarray, w: np.ndarray, b: np.ndarray):
    """Host-side prep shared by kernel() and the traced test path."""
    bf = ml_dtypes.bfloat16
    f8 = ml_dtypes.float8_e4m3
    x = np.ascontiguousarray(inputs.reshape(B_FULL, T), dtype=np.float32)
    with_bias = bool(np.any(b))
    wb = np.ascontiguousarray(w).astype(bf)
    bb = np.ascontiguousarray(b).astype(bf) if with_bias else None

    # Exact-ish fp8 Dekker pair: hi + lo == x to ~8e-4 (fp32 sum).
    hi = x.astype(f8)
    lo = (x - hi.astype(np.float32)).astype(f8)
    x2 = np.stack([hi, lo], axis=1).reshape(B_FULL, 2 * T)

    sel = np.zeros((BS, BS, 2, P), dtype=f8)
    for bi in range(BS):
        sel[bi, bi, :, :] = 1.0
    sel = sel.reshape(BS, BS * 2 * P)

    nc = _get_nc(with_bias)
    in_maps = []
    for c in range(N_CORES):
        m = {
            "x2": np.ascontiguousarray(x2[c * BS : (c + 1) * BS]),
            "sel2": sel,
            "w": wb,
        }
        if with_bias:
            m["bvec"] = bb
        in_maps.append(m)
    return nc, in_maps


def _finish(res) -> np.ndarray:
    out = np.concatenate([np.asarray(r["out"]) for r in res.results], axis=0)
    return out.astype(np.float32)


def kernel(inputs: np.ndarray, w: np.ndarray, b: np.ndarray, **kw) -> np.ndarray:
    nc, in_maps = _prepare(inputs, w, b)
    res = run_bass_kernel_spmd(nc, in_maps, core_ids=list(range(N_CORES)))
    return _finish(res)
